# revision 23
# baseline (speedup 1.0000x reference)
"""GQA attention kernel for 8 Trainium2 NeuronCores — v4.

Sharding: core c = 4*b + h handles batch b (of 2) and kv-head h (of 4),
i.e. one kv head + its 4 grouped query heads. Each core computes its head
group's partial contribution to the output projection; the host sums the
4 partials per batch. No collectives.

The PE is the global bottleneck (~1170 N=512 matmuls; the sustained PE
clock is 2.4 GHz in the chip's fast power state, 2.0 GHz in the slow one
— both observed run-to-run). The schedule is built so the PE never
idles in either state:

- phase 1: K/V projections over all 4 S-bands + Q projections of band 0,
  V^T transposed to V in bf16. 12 warm-up matmuls on an identity tile
  (no DMA dependency) fill the ~8us DMA startup hole and get the PE past
  the HAM cold window; wk is DMA'd in 4 chunk-quarters interleaved with
  the x band-0 quarters so real matmuls start on the first quarter.
- Q projections of bands 1-3 are deferred and interleaved one
  contraction-chunk matmul per sk into attention qtiles 0-2; the
  out-projection of qt0 interleaves into qt3 the same way. Every
  attention iteration therefore carries 3 PE matmuls (score, AV, fill)
  per 128-key block.
- Scores for two consecutive key blocks land in one 2-bank PSUM pair and
  are EXP'd by a single [128,1024] ACT instruction: per sk-pair the ACT
  does ~1.12us vs the PE's 6 matmuls (1.30us @2.4GHz) — PE-bound with
  margin, and exp latency is hidden by trailing the AV matmuls one pair
  behind the scores.
- Out-projections of qt1-3 run as a dense pure-PE tail (ACT idle there
  is free — total PE work is what it is). Output partials are bf16,
  DMA'd per 512-column chunk as soon as each is copied out of PSUM.
- Single PSUM pool for the whole kernel: sc (2-bank score pair) x2 +
  py x2 + aux x2 = 8 banks. aux rotates {warmup, deferred-Q
  accumulators, outproj chunks, softmax-denominator matmuls}; the
  phase-1 V-transposes share the (then unused) py tag. The softmax
  denominator accumulates in two independent chains (DVE even pairs,
  GpSimd odd pairs) folded per pass, and qt3's outproj copies also run
  on GpSimd, keeping every non-PE engine under ~90% of the PE pass time
  in both clock states.

Device math per core (S=2048, H=2048, d=128):
  QT_g = (x @ Wq_g + bq_g)^T          [d, S]   g=0..3   (bf16)
  KT   = (x @ Wk_h)^T                 [d, S]            (bk cancels in softmax)
  V    = x @ Wv_h                     [S, d]   (computed as V^T then PE-transposed)
  S^T  = KT^T-blocks @ QT             [Sk, Sq]
  P^T  = exp(SCALE * S^T)             (bf16, no max-subtraction: |s| <~ 6)
  den  = ones.T @ (DVE-sum of P^T tiles)
  y^T  = V^T-blocks.T @ P^T (PSUM accum);  yT := y^T * (1/den)
  out += yT_g^T @ Wo_g                [S, H]  (partial over this head group)
Host: out[b] = sum_h partial + (bv_rep @ Wo + bo).
"""

import numpy as np
import ml_dtypes

B = 2
S = 2048
HIDDEN = 2048
NKV = 4
GROUP = 4
D = 128
SCALE = D ** -0.5

BAND = 512            # S-columns per projection band
NBAND = S // BAND     # 4
NCH = HIDDEN // 128   # 16 contraction chunks
QTILE = 512           # queries per attention tile
NQT = S // QTILE      # 4
NSK = S // 128        # 16 key tiles
NSKP = NSK // 2       # 8 sk pairs
NWARM = 12            # PE warmup matmuls during DMA startup

_CACHE = {}
LAST_RESULTS = None
TRACE = False
TMPDIR = None


def _build():
    import concourse.bass as bass
    import concourse.bacc as bacc
    import concourse.mybir as mybir
    import concourse.tile as tile
    from concourse.masks import make_identity

    f32 = mybir.dt.float32
    bf16 = mybir.dt.bfloat16
    EXP = mybir.ActivationFunctionType.Exp
    IDENT = mybir.ActivationFunctionType.Identity
    COPY = mybir.ActivationFunctionType.Copy

    nc = bacc.Bacc(trn_type="TRN2", target_bir_lowering=False, debug=False)

    xT = nc.dram_tensor("xT", [NBAND, 128, NCH, BAND], bf16, kind="ExternalInput").ap()
    wq = nc.dram_tensor("wq", [GROUP, 128, NCH, 128], bf16, kind="ExternalInput").ap()
    wk = nc.dram_tensor("wk", [128, NCH, 128], bf16, kind="ExternalInput").ap()
    wv = nc.dram_tensor("wv", [128, NCH, 128], bf16, kind="ExternalInput").ap()
    wo = nc.dram_tensor("wo", [GROUP, 128, HIDDEN], bf16, kind="ExternalInput").ap()
    bq = nc.dram_tensor("bq", [128, GROUP], f32, kind="ExternalInput").ap()
    onesk = nc.dram_tensor("onesk", [128, 128], bf16, kind="ExternalInput").ap()
    out = nc.dram_tensor("out", [S, HIDDEN], bf16, kind="ExternalOutput").ap()

    with tile.TileContext(nc) as tc:
        with (
            tc.tile_pool(name="const", bufs=1) as constp,
            tc.tile_pool(name="wts", bufs=1) as wtsp,
            tc.tile_pool(name="xb", bufs=1) as xbp,
            tc.tile_pool(name="qkv", bufs=1) as qkvp,
            tc.tile_pool(name="ptbuf", bufs=2) as ptp,
            tc.tile_pool(name="rbuf", bufs=2) as rp,
            tc.tile_pool(name="dens", bufs=2) as densp,
            tc.tile_pool(name="ytbuf", bufs=16) as ytp,
            tc.tile_pool(name="outbuf", bufs=2) as outp,
            tc.tile_pool(name="ps", bufs=1, space="PSUM") as psp,
        ):
            # PSUM tags: sc (2-bank score pair) x2 + py x2 + aux x2 = 8 banks
            def sc_tile():
                return psp.tile([128, 2 * QTILE], f32, name="sc", tag="sc", bufs=2)

            def aux_tile(name):
                return psp.tile([128, QTILE], f32, name=name, tag="aux", bufs=2)

            # ---- constants that need no DMA (feed the PE warmup) ----
            ident = constp.tile([128, 128], f32, name="ident")
            make_identity(nc, ident[:, :])
            ident_bf = constp.tile([128, 128], bf16, name="ident_bf")
            nc.vector.tensor_copy(ident_bf[:, :], ident[:, :])
            wrhs = constp.tile([128, BAND], bf16, name="wrhs")
            nc.vector.memset(wrhs[:, :], 0.0)

            # ---- DMAs in consumption order ----
            onesk_t = constp.tile([128, 128], bf16, name="onesk_t")
            nc.sync.dma_start(out=onesk_t[:, :], in_=onesk)
            bq_t = constp.tile([128, GROUP], f32, name="bq_t")
            nc.sync.dma_start(out=bq_t[:, :], in_=bq)

            # interleave wk quarters with x band-0 quarters so the first
            # K-projection matmuls start as soon as the first pair lands
            wk_t = wtsp.tile([128, NCH, 128], bf16, name="wk_t")
            b0q = []
            for c4 in range(4):
                nc.sync.dma_start(
                    out=wk_t[:, 4 * c4:4 * c4 + 4, :], in_=wk[:, 4 * c4:4 * c4 + 4, :]
                )
                t = xbp.tile([128, 4, BAND], bf16, name=f"b0q{c4}", tag="band0q",
                             bufs=4)
                nc.sync.dma_start(out=t[:, :, :], in_=xT[0, :, 4 * c4:4 * c4 + 4, :])
                b0q.append(t)

            wv_t = wtsp.tile([128, NCH, 128], bf16, name="wv_t")
            nc.sync.dma_start(out=wv_t[:, :, :], in_=wv)
            wq_t = []
            for g in range(GROUP):
                t = wtsp.tile([128, NCH, 128], bf16, name=f"wq_t{g}", tag=f"wq{g}")
                nc.sync.dma_start(out=t[:, :, :], in_=wq[g])
                wq_t.append(t)

            bands = [None] * NBAND
            for bd in range(1, NBAND):
                bands[bd] = xbp.tile(
                    [128, NCH, BAND], bf16, name=f"band{bd}", tag=f"band{bd}"
                )
                nc.sync.dma_start(out=bands[bd][:, :, :], in_=xT[bd])

            wo_t = []
            for g in range(GROUP):
                t = wtsp.tile([128, HIDDEN], bf16, name=f"wo_t{g}", tag=f"wo{g}")
                nc.sync.dma_start(out=t[:, :], in_=wo[g])
                wo_t.append(t)

            # ---- persistent activations ----
            qt_t = []
            for g in range(GROUP):
                t = qkvp.tile([128, S], bf16, name=f"qt{g}", tag=f"qt{g}")
                qt_t.append(t)
            kt_t = qkvp.tile([128, S], bf16, name="kt_t")
            v_t = qkvp.tile([128, S], bf16, name="v_t")
            vt_b = qkvp.tile([128, S], bf16, name="vt_b")

            def bch(bd, c):
                if bd == 0:
                    return b0q[c // 4][:, c % 4, :]
                return bands[bd][:, c, :]

            # ====== phase 1: K/V all bands + Q band 0 ======
            # PE warmup: no data dependencies; fills the DMA startup hole
            # and gets HAM past the cold window
            pwarm = aux_tile("pwarm")
            for _ in range(NWARM):
                nc.tensor.matmul(
                    out=pwarm[:, :], lhsT=ident_bf[:, :], rhs=wrhs[:, :],
                    start=True, stop=True,
                )

            def transpose_v(sk, evac="scalar"):
                # V^T 128-block -> V block (bf16 transpose, 1 cyc/row);
                # ptr shares the attention py tag (unused during phase 1)
                ptr = psp.tile([128, 128], bf16, name="ptr", tag="py", bufs=2)
                nc.tensor.transpose(
                    ptr[:, :], vt_b[:, sk * 128:(sk + 1) * 128], ident_bf[:, :]
                )
                eng = nc.scalar if evac == "scalar" else nc.vector
                if evac == "scalar":
                    eng.activation(
                        v_t[:, sk * 128:(sk + 1) * 128], ptr[:, :], COPY
                    )
                else:
                    eng.tensor_copy(v_t[:, sk * 128:(sk + 1) * 128], ptr[:, :])

            for bd in range(NBAND):
                bsl = slice(bd * BAND, (bd + 1) * BAND)

                # K^T accumulation; transposes of the previous band's V^T
                # interleave every 4 chunks so their PSUM-evac copies have
                # time to drain between PE uses of the 2-buf rotation
                pk = sc_tile()
                for c in range(NCH):
                    nc.tensor.matmul(
                        out=pk[:, 0:BAND],
                        lhsT=wk_t[:, c, :],
                        rhs=bch(bd, c),
                        start=(c == 0), stop=(c == NCH - 1),
                    )
                    if bd == 0 and c < 12:
                        # extra warmups between DMA-paced band-0 chunks so
                        # the PE never idles long enough to re-throttle
                        nc.tensor.matmul(
                            out=pwarm[:, :], lhsT=ident_bf[:, :],
                            rhs=wrhs[:, :], start=True, stop=True,
                        )
                    if bd >= 2 and c % 4 == 3:
                        transpose_v((bd - 1) * 4 + c // 4)
                nc.scalar.activation(kt_t[:, bsl], pk[:, 0:BAND], COPY)

                # V^T accumulation
                pv = sc_tile()
                for c in range(NCH):
                    nc.tensor.matmul(
                        out=pv[:, 0:BAND],
                        lhsT=wv_t[:, c, :],
                        rhs=bch(bd, c),
                        start=(c == 0), stop=(c == NCH - 1),
                    )
                    if bd == 0 and c < 6:
                        nc.tensor.matmul(
                            out=pwarm[:, :], lhsT=ident_bf[:, :],
                            rhs=wrhs[:, :], start=True, stop=True,
                        )
                nc.scalar.activation(vt_b[:, bsl], pv[:, 0:BAND], COPY)

                # Q^T per local head, band 0 only, with band-0 V transposes
                # interleaved between head blocks
                if bd == 0:
                    for g in range(GROUP):
                        pq = sc_tile()
                        for c in range(NCH):
                            nc.tensor.matmul(
                                out=pq[:, 0:BAND],
                                lhsT=wq_t[g][:, c, :],
                                rhs=bch(0, c),
                                start=(c == 0), stop=(c == NCH - 1),
                            )
                        nc.scalar.activation(
                            qt_t[g][:, bsl], pq[:, 0:BAND], IDENT,
                            bias=bq_t[:, g:g + 1],
                        )
                        transpose_v(g)
            # band 3's V transposes are emitted as qt0-g0 preamble fillers
            # (DVE-evacuated so the first exps aren't queued behind copies)

            # ====== phase 2: attention w/ interleaved filler + outproj ======
            yt_all = {}
            pending = [None]     # deferred den+normalize closure
            pending_av = [None]  # previous pass's trailing AV pair

            def flush_pending():
                if pending[0] is not None:
                    pending[0]()
                    pending[0] = None

            def flush_av():
                if pending_av[0] is not None:
                    pending_av[0]()
                    pending_av[0] = None

            def attn_group(qt, g, fillers=(), tail_fillers=(), pre_fillers=()):
                """One head-group of attention over qtile qt.

                Scores for sk-pair p land in one 2-bank PSUM pair, EXP'd by
                a single [128,1024] ACT op. AV matmuls trail one pair; the
                last pair's AVs carry over into the NEXT group (pending_av)
                so the final exp's latency is never exposed at a pass end.
                fillers: callbacks fill(sk) issuing exactly one PE matmul
                each, called once per sk; used for deferred Q-band
                projections or interleaved out-projection chunks.
                pre_fillers: one callback per early pair index (extra PE op).
                """
                qsl = slice(qt * QTILE, (qt + 1) * QTILE)
                py = psp.tile([128, QTILE], f32, name="py", tag="py", bufs=2)
                pt_all = ptp.tile([128, NSK * QTILE], bf16, name="pt", tag="pt")

                def pt(a, b=None):
                    return pt_all[:, a * QTILE:(b or a + 1) * QTILE]

                # softmax-denominator accumulation: two independent bf16
                # chains (even pairs on DVE, odd pairs on GpSimd), folded at
                # the end of the pass — keeps both engines under ~60% busy
                Rd = rp.tile([128, QTILE], bf16, name="rd", tag="rd")
                Rp = rp.tile([128, QTILE], bf16, name="rp_", tag="rp_")

                for p in range(NSKP):
                    sk0, sk1 = 2 * p, 2 * p + 1
                    ps2 = sc_tile()
                    for h, sk in ((0, sk0), (1, sk1)):
                        nc.tensor.matmul(
                            out=ps2[:, h * QTILE:(h + 1) * QTILE],
                            lhsT=kt_t[:, sk * 128:(sk + 1) * 128],
                            rhs=qt_t[g][:, qsl],
                            start=True, stop=True,
                        )
                    nc.scalar.activation(
                        pt(sk0, sk1 + 1), ps2[:, :], EXP, scale=SCALE
                    )
                    if p == 1:
                        flush_av()
                    if p < len(pre_fillers):
                        pre_fillers[p]()
                    for fill in fillers:
                        fill(sk0)
                        fill(sk1)
                    # den accumulation: GpSimd (slow, ~1us/add) takes only
                    # the early pairs 1 and 3 so nothing near the pass end
                    # ever waits on it; DVE takes the rest and folds the
                    # GpSimd chain in mid-pass
                    eng, R = (nc.gpsimd, Rp) if p in (1, 3) else (nc.vector, Rd)
                    if p < 2:
                        eng.tensor_add(R[:, :], pt(sk0), pt(sk1))
                    else:
                        eng.tensor_add(R[:, :], R[:, :], pt(sk0))
                        eng.tensor_add(R[:, :], R[:, :], pt(sk1))
                    if p == 5:
                        nc.vector.tensor_add(Rd[:, :], Rd[:, :], Rp[:, :])
                    if p >= 1:
                        # AV trails scores by one pair so exp latency and
                        # jitter stay hidden
                        for psk in (sk0 - 2, sk1 - 2):
                            nc.tensor.matmul(
                                out=py[:, :],
                                lhsT=v_t[:, psk * 128:(psk + 1) * 128],
                                rhs=pt(psk),
                                start=(psk == 0), stop=False,
                            )
                    if p == 1:
                        flush_pending()
                for tf in tail_fillers:
                    tf()

                def carry_av(py=py, pt_all=pt_all):
                    for psk in (NSK - 2, NSK - 1):
                        nc.tensor.matmul(
                            out=py[:, :],
                            lhsT=v_t[:, psk * 128:(psk + 1) * 128],
                            rhs=pt_all[:, psk * QTILE:(psk + 1) * QTILE],
                            start=False, stop=(psk == NSK - 1),
                        )

                pending_av[0] = carry_av

                def normalize(qt=qt, g=g, py=py, R=Rd):
                    # all-ones stationary: out[i, q] = sum_p R[p, q] —
                    # partition-reduces AND replicates den on all 128
                    # partitions in one matmul
                    pden = aux_tile("pden")
                    nc.tensor.matmul(
                        out=pden[:, :],
                        lhsT=onesk_t[:, :],
                        rhs=R[:, :],
                        start=True, stop=True,
                    )
                    rb = densp.tile([128, QTILE], f32, name="rb", tag="rb")
                    nc.vector.reciprocal_approx_fast(rb[:, :], pden[:, :])
                    yt = ytp.tile([128, QTILE], bf16, name="yt", tag="yt")
                    nc.vector.tensor_mul(yt[:, :], py[:, :], rb[:, :])
                    yt_all[(qt, g)] = yt

                pending[0] = normalize

            def make_qproj_filler(bd, g):
                """Deferred Q-projection of (band bd, head g): one
                contraction-chunk matmul per sk into an aux psum bank,
                evacuated with the bias add at the end of the pass."""
                qacc = aux_tile(f"qacc{bd}")
                bsl = slice(bd * BAND, (bd + 1) * BAND)

                def fill(sk):
                    nc.tensor.matmul(
                        out=qacc[:, :],
                        lhsT=wq_t[g][:, sk, :],
                        rhs=bands[bd][:, sk, :],
                        start=(sk == 0), stop=(sk == NCH - 1),
                    )

                def evac():
                    nc.scalar.activation(
                        qt_t[g][:, bsl], qacc[:, :], IDENT,
                        bias=bq_t[:, g:g + 1],
                    )

                return fill, evac

            def outproj_block(proj_qt, i, outs, j, po):
                """Four head-accumulation matmuls for out-projection chunk
                (proj_qt, i, j), then copy + DMA of the 512-col chunk."""
                for gp in range(GROUP):
                    nc.tensor.matmul(
                        out=po[:, :],
                        lhsT=yt_all[(proj_qt, gp)][:, i * 128:(i + 1) * 128],
                        rhs=wo_t[gp][:, j * 512:(j + 1) * 512],
                        start=(gp == 0), stop=(gp == GROUP - 1),
                    )
                csl = slice(j * 512, (j + 1) * 512)
                nc.vector.tensor_copy(outs[:, csl], po[:, :])
                r0 = proj_qt * QTILE + i * 128
                nc.sync.dma_start(
                    out=out[r0:r0 + 128, j * 512:(j + 1) * 512],
                    in_=outs[:, csl],
                )

            def make_outproj_filler(proj_qt, i):
                """Out-projection block (proj_qt, i) interleaved one matmul
                per sk: chunk j = sk//4 accumulated over heads gp = sk%4."""
                outs = outp.tile([128, HIDDEN], bf16, name="outs", tag="outs")
                state = {}

                def fill(sk):
                    j, gp = sk // 4, sk % 4
                    if gp == 0:
                        state["po"] = aux_tile("po")
                    po = state["po"]
                    nc.tensor.matmul(
                        out=po[:, :],
                        lhsT=yt_all[(proj_qt, gp)][:, i * 128:(i + 1) * 128],
                        rhs=wo_t[gp][:, j * 512:(j + 1) * 512],
                        start=(gp == 0), stop=(gp == GROUP - 1),
                    )
                    if gp == GROUP - 1:
                        # DVE copy (GpSimd cannot read PSUM); DVE has room
                        # in qt3 because half the den-adds live on GpSimd
                        csl = slice(j * 512, (j + 1) * 512)
                        nc.vector.tensor_copy(outs[:, csl], po[:, :])
                        r0 = proj_qt * QTILE + i * 128
                        nc.sync.dma_start(
                            out=out[r0:r0 + 128, j * 512:(j + 1) * 512],
                            in_=outs[:, csl],
                        )

                return fill

            # qt0-2: + deferred Q-projection of bands 1-3; qt3: + outproj(qt0)
            for qt in range(3):
                for g in range(GROUP):
                    f, e = make_qproj_filler(qt + 1, g)
                    pre = ()
                    if qt == 0 and g == 0:
                        # band-3 V transposes ride the first four pairs
                        pre = tuple(
                            (lambda t=t: transpose_v(12 + t, evac="vector"))
                            for t in range(4)
                        )
                    attn_group(qt, g, fillers=(f,), tail_fillers=(e,),
                               pre_fillers=pre)
            for g in range(GROUP):
                attn_group(3, g, fillers=(make_outproj_filler(0, g),))

            # tail: out-projection of qt1-3, pure PE. The first block is
            # emitted BEFORE the final carry/normalize flushes so the PE
            # keeps streaming while the last pass's exp drains.
            def tail_block(proj_qt, i):
                outs = outp.tile([128, HIDDEN], bf16, name="outs", tag="outs")
                for j in range(4):
                    po = aux_tile("po")
                    outproj_block(proj_qt, i, outs, j, po)

            tail_block(1, 0)
            flush_av()
            flush_pending()
            for i in range(1, QTILE // 128):
                tail_block(1, i)
            for proj_qt in (2, 3):
                for i in range(QTILE // 128):
                    tail_block(proj_qt, i)

    nc.finalize()
    return nc


def _get_nc():
    if "nc" not in _CACHE:
        _CACHE["nc"] = _build()
    return _CACHE["nc"]


def kernel(x, Wq, bq, Wk, bk, Wv, bv, Wo, bo):
    global LAST_RESULTS
    from concourse.bass_utils import run_bass_kernel_spmd

    bf = ml_dtypes.bfloat16
    x = np.asarray(x, np.float32)
    Wq = np.asarray(Wq, np.float32)
    Wk = np.asarray(Wk, np.float32)
    Wv = np.asarray(Wv, np.float32)
    Wo = np.asarray(Wo, np.float32)
    bq = np.asarray(bq, np.float32)
    bv = np.asarray(bv, np.float32)
    bo = np.asarray(bo, np.float32)

    nc = _get_nc()

    onesk_np = np.ones((128, 128), bf)

    in_maps = []
    for c in range(8):
        b, h = divmod(c, NKV)
        xTb = x[b].T  # [HIDDEN, S]
        xTh = np.ascontiguousarray(
            xTb.reshape(NCH, 128, NBAND, BAND).transpose(2, 1, 0, 3)
        ).astype(bf)
        # wq[g]: [128, NCH, 128] per local head
        wqh = np.ascontiguousarray(
            Wq[:, h * 512:(h + 1) * 512]
            .reshape(NCH, 128, GROUP, 128).transpose(2, 1, 0, 3)
        ).astype(bf)
        wkh = np.ascontiguousarray(
            Wk[:, h * 128:(h + 1) * 128].reshape(NCH, 128, 128).transpose(1, 0, 2)
        ).astype(bf)
        wvh = np.ascontiguousarray(
            Wv[:, h * 128:(h + 1) * 128].reshape(NCH, 128, 128).transpose(1, 0, 2)
        ).astype(bf)
        woh = np.ascontiguousarray(
            Wo[h * 512:(h + 1) * 512, :].reshape(GROUP, 128, HIDDEN)
        ).astype(bf)
        bqh = np.ascontiguousarray(
            bq[h * 512:(h + 1) * 512].reshape(GROUP, 128).T
        )
        in_maps.append({
            "xT": xTh, "wq": wqh, "wk": wkh, "wv": wvh, "wo": woh,
            "bq": bqh, "onesk": onesk_np,
        })

    res = run_bass_kernel_spmd(
        nc, in_maps, list(range(8)), trace=TRACE, tmpdir=TMPDIR
    )
    LAST_RESULTS = res

    # host-side constant bias: (bv repeated per head group) @ Wo + bo
    bv_rep = np.broadcast_to(
        bv.reshape(NKV, 1, D), (NKV, GROUP, D)
    ).reshape(HIDDEN)
    bias_row = bv_rep @ Wo + bo  # [HIDDEN]

    out = np.empty((B, S, HIDDEN), np.float32)
    for b in range(B):
        acc = res.results[b * NKV + 0]["out"].astype(np.float32)
        for h in range(1, NKV):
            acc = acc + res.results[b * NKV + h]["out"].astype(np.float32)
        out[b] = acc + bias_row
    return out
